# revision 2
# baseline (speedup 1.0000x reference)
"""Bass/Trainium2 kernel for nn_ClassQueryAttention.

Math (per batch b, x flattened to [C=256, N=16384]):
  logits[k,n] = (qe @ Wk) @ x / sqrt(D)          (per-k bias qe@bk cancels in softmax)
  p = exp(logits)  (no max-subtraction needed: logits ~ N(0,1))
  s_k = sum_n p[k,n];  r_k = 1/s_k
  y[k,c] = sum_n p[k,n] x[c,n]                   (flash-style, accumulated in PSUM)
  xa[c]  = sum_k r_k y[k,c]
  gate   = (Wo@Wv) @ xa + K*(Wo@bv + bo)
  out[c,n] = x[c,n] * gate[c]

Sharding: data-parallel over batch B=8, one batch per NeuronCore, no collectives.
Per-core HBM traffic: 2 reads of x (16 MiB each) + 1 write (16 MiB) = 48 MiB.
"""

import sys
from contextlib import ExitStack

import numpy as np

sys.path.insert(0, "/opt/trn_rl_repo")

import concourse.bass as bass  # noqa: E402
import concourse.tile as tile  # noqa: E402
from concourse import bacc, mybir  # noqa: E402
from concourse.bass_utils import run_bass_kernel_spmd  # noqa: E402

B, C, HW = 8, 256, 128 * 128
K, D = 21, 256
P = 128          # partition count / channel chunk
NB = 2048        # DMA big-tile pixels
NQ = 512         # logits quad pixels
NS = 128         # transpose subtile pixels
F32 = mybir.dt.float32
AF = mybir.ActivationFunctionType


def _body(ctx: ExitStack, tc: tile.TileContext, x, qk, m2, g0, ident, out):
    nc = tc.nc

    consts = ctx.enter_context(tc.tile_pool(name="consts", bufs=1))
    qk0 = consts.tile([P, K], F32, tag="qk0")
    qk1 = consts.tile([P, K], F32, tag="qk1")
    m2t0 = consts.tile([P, C], F32, tag="m2t0")
    m2t1 = consts.tile([P, C], F32, tag="m2t1")
    g0_sb = consts.tile([P, 2], F32, tag="g0")
    id_sb = consts.tile([P, P], F32, tag="ident")
    s_acc = consts.tile([K, HW // NQ], F32, tag="s_acc")

    nc.sync.dma_start(qk0[:], qk[0:P, :])
    nc.sync.dma_start(qk1[:], qk[P : 2 * P, :])
    nc.sync.dma_start(m2t0[:], m2[0:P, :])
    nc.sync.dma_start(m2t1[:], m2[P : 2 * P, :])
    nc.sync.dma_start(g0_sb[:], g0[:, :])
    nc.sync.dma_start(id_sb[:], ident[:, :])

    xbig = ctx.enter_context(tc.tile_pool(name="xbig", bufs=3))
    ps_small = ctx.enter_context(tc.tile_pool(name="ps_small", bufs=3, space="PSUM"))
    ps_xt = ctx.enter_context(tc.tile_pool(name="ps_xt", bufs=2, space="PSUM"))
    ps_y = ctx.enter_context(tc.tile_pool(name="ps_y", bufs=1, space="PSUM"))
    sb_xt = ctx.enter_context(tc.tile_pool(name="sb_xt", bufs=3))
    sb_pt = ctx.enter_context(tc.tile_pool(name="sb_pt", bufs=3))
    sb_p = ctx.enter_context(tc.tile_pool(name="sb_p", bufs=3))
    xcpool = ctx.enter_context(tc.tile_pool(name="xc", bufs=6))

    # ---------------- Phase A: stream x, build p, accumulate y and s ----------
    y_ps = ps_y.tile([K, C], F32, tag="y")
    n_big = HW // NB                 # 8
    n_quad = NB // NQ                # 4
    n_sub = NQ // NS                 # 4
    idx, last = 0, (HW // NS) - 1    # 128 y-matmuls
    for bt in range(n_big):
        xb0 = xbig.tile([P, NB], F32, tag="xb0")
        xb1 = xbig.tile([P, NB], F32, tag="xb1")
        nc.sync.dma_start(xb0[:], x[0:P, bt * NB : (bt + 1) * NB])
        nc.sync.dma_start(xb1[:], x[P : 2 * P, bt * NB : (bt + 1) * NB])
        for q in range(n_quad):
            t = bt * n_quad + q
            sl = slice(q * NQ, (q + 1) * NQ)
            l_ps = ps_small.tile([K, NQ], F32, tag="pssmall")
            nc.tensor.matmul(l_ps[:], qk0[:], xb0[:, sl], start=True, stop=False)
            nc.tensor.matmul(l_ps[:], qk1[:], xb1[:, sl], start=False, stop=True)
            p_sb = sb_p.tile([K, NQ], F32, tag="p")
            nc.scalar.activation(
                p_sb[:], l_ps[:], AF.Exp, accum_out=s_acc[:, t : t + 1]
            )
            pt_ps = ps_small.tile([P, n_sub * K], F32, tag="pssmall")
            for j in range(n_sub):
                nc.tensor.transpose(
                    pt_ps[:, j * K : (j + 1) * K],
                    p_sb[:, j * NS : (j + 1) * NS],
                    id_sb[0:K, 0:K],
                )
            pt_sb = sb_pt.tile([P, n_sub * K], F32, tag="ptsb")
            nc.vector.tensor_copy(pt_sb[:], pt_ps[:])

            xt_ps = ps_xt.tile([P, n_sub, C], F32, tag="xt")
            for j in range(n_sub):
                ss = slice(q * NQ + j * NS, q * NQ + (j + 1) * NS)
                nc.tensor.transpose(xt_ps[:, j, 0:P], xb0[:, ss], id_sb[:, :])
                nc.tensor.transpose(xt_ps[:, j, P : 2 * P], xb1[:, ss], id_sb[:, :])
            xt_sb = sb_xt.tile([P, n_sub, C], F32, tag="xtsb")
            nc.vector.tensor_copy(xt_sb[:], xt_ps[:])

            for j in range(n_sub):
                nc.tensor.matmul(
                    y_ps[:],
                    pt_sb[:, j * K : (j + 1) * K],
                    xt_sb[:, j, :],
                    start=(idx == 0),
                    stop=(idx == last),
                    skip_group_check=True,
                )
                idx += 1

    # ---------------- Phase B: softmax denominators -> xa -> gate -------------
    s_sb = consts.tile([K, 1], F32, tag="s_sb")
    nc.vector.reduce_sum(s_sb[:], s_acc[:], axis=mybir.AxisListType.X)
    r_sb = consts.tile([K, 1], F32, tag="r_sb")
    nc.vector.reciprocal(r_sb[:], s_sb[:])
    y_sb = consts.tile([K, C], F32, tag="y_sb")
    nc.vector.tensor_copy(y_sb[:], y_ps[:])

    xa_ps = ps_small.tile([1, C], F32, tag="pssmall")
    nc.tensor.matmul(xa_ps[:], r_sb[:], y_sb[:], start=True, stop=True)
    xa_sb = consts.tile([1, C], F32, tag="xa_sb")
    nc.vector.tensor_copy(xa_sb[:], xa_ps[:])

    xat_ps = ps_small.tile([P, 2], F32, tag="pssmall")
    for j in range(2):
        nc.tensor.transpose(
            xat_ps[:, j : j + 1], xa_sb[0:1, j * P : (j + 1) * P], id_sb[0:1, 0:1]
        )
    xat_sb = consts.tile([P, 2], F32, tag="xat_sb")
    nc.vector.tensor_copy(xat_sb[:], xat_ps[:])

    gate_ps = ps_small.tile([P, 2], F32, tag="pssmall")
    for cc in range(2):
        csl = slice(cc * P, (cc + 1) * P)
        nc.tensor.matmul(
            gate_ps[:, cc : cc + 1], m2t0[:, csl], xat_sb[:, 0:1],
            start=True, stop=False, skip_group_check=True,
        )
        nc.tensor.matmul(
            gate_ps[:, cc : cc + 1], m2t1[:, csl], xat_sb[:, 1:2],
            start=False, stop=True, skip_group_check=True,
        )
    gate_sb = consts.tile([P, 2], F32, tag="gate_sb")
    nc.vector.tensor_add(gate_sb[:], gate_ps[:], g0_sb[:])

    # ---------------- Phase C: out = x * gate ---------------------------------
    for cc in range(2):
        csl = slice(cc * P, (cc + 1) * P)
        for nt in range(HW // NB):
            xc = xcpool.tile([P, NB], F32, tag="xc")
            nsl = slice(nt * NB, (nt + 1) * NB)
            nc.sync.dma_start(xc[:], x[csl, nsl])
            nc.vector.tensor_scalar_mul(xc[:], xc[:], gate_sb[:, cc : cc + 1])
            nc.sync.dma_start(out[csl, nsl], xc[:])


def build_nc():
    nc = bacc.Bacc(
        "TRN2",
        target_bir_lowering=False,
        debug=False,
        enable_asserts=False,
        num_devices=B,
    )
    x = nc.dram_tensor("x", [C, HW], F32, kind="ExternalInput").ap()
    qk = nc.dram_tensor("qkT", [C, K], F32, kind="ExternalInput").ap()
    m2 = nc.dram_tensor("m2t", [C, C], F32, kind="ExternalInput").ap()
    g0 = nc.dram_tensor("g0", [P, 2], F32, kind="ExternalInput").ap()
    ident = nc.dram_tensor("ident", [P, P], F32, kind="ExternalInput").ap()
    out = nc.dram_tensor("out", [C, HW], F32, kind="ExternalOutput").ap()

    with tile.TileContext(nc) as tc:
        with ExitStack() as ctx:
            _body(ctx, tc, x, qk, m2, g0, ident, out)
    nc.compile()
    return nc


_NC = None


def _get_nc():
    global _NC
    if _NC is None:
        _NC = build_nc()
    return _NC


def make_in_maps(x, query_embed, Wk, bk, Wv, bv, Wo, bo):
    x = np.asarray(x, dtype=np.float32)
    qe = np.asarray(query_embed, dtype=np.float64)
    Wk64 = np.asarray(Wk, dtype=np.float64)
    Wv64 = np.asarray(Wv, dtype=np.float64)
    Wo64 = np.asarray(Wo, dtype=np.float64)
    bv64 = np.asarray(bv, dtype=np.float64)
    bo64 = np.asarray(bo, dtype=np.float64)

    qkT = ((qe @ Wk64) / np.sqrt(float(D))).T.astype(np.float32).copy()
    m2t = (Wo64 @ Wv64).T.astype(np.float32).copy()
    g0 = (float(K) * (Wo64 @ bv64 + bo64)).astype(np.float32)
    g0c = np.ascontiguousarray(g0.reshape(2, P).T)
    ident = np.eye(P, dtype=np.float32)

    return [
        {
            "x": np.ascontiguousarray(x[b].reshape(C, HW)),
            "qkT": qkT,
            "m2t": m2t,
            "g0": g0c,
            "ident": ident,
        }
        for b in range(B)
    ]


def kernel(x, query_embed, Wk, bk, Wv, bv, Wo, bo, _trace=False, **kw):
    in_maps = make_in_maps(x, query_embed, Wk, bk, Wv, bv, Wo, bo)
    nc = _get_nc()
    res = run_bass_kernel_spmd(nc, in_maps, core_ids=list(range(B)), trace=_trace, **kw)
    out = np.stack(
        [res.results[b]["out"].reshape(C, 128, 128) for b in range(B)]
    ).astype(np.float32)
    if _trace:
        kernel.last_results = res
    return out


# revision 7
# speedup vs baseline: 16.6609x; 16.6609x over previous
"""Bass/Trainium2 kernel for nn_ClassQueryAttention.

Math (per batch b, x flattened to [C=256, N=16384]):
  logits[k,n] = (qe @ Wk) @ x / sqrt(D)          (per-k bias qe@bk cancels in softmax)
  p = exp(logits)  (no max-subtraction needed: logits ~ N(0,1))
  s_k = sum_n p[k,n];  r_k = 1/s_k
  y[k,c] = sum_n p[k,n] x[c,n]                   (flash-style, accumulated in PSUM)
  xa[c]  = sum_k r_k y[k,c]
  gate   = (Wo@Wv) @ xa + K*(Wo@bv + bo)
  out[c,n] = x[c,n] * gate[c]

Sharding: data-parallel over batch B=8, one batch per NeuronCore, no collectives.
Per-core HBM traffic: 2 reads of x (16 MiB each) + 1 write (16 MiB) = 48 MiB.
"""

import sys
from contextlib import ExitStack

import numpy as np

sys.path.insert(0, "/opt/trn_rl_repo")

import concourse.bass as bass  # noqa: E402
import concourse.tile as tile  # noqa: E402
from concourse import bacc, mybir  # noqa: E402
from concourse.bass_utils import run_bass_kernel_spmd  # noqa: E402

B, C, HW = 8, 256, 128 * 128
K, D = 21, 256
P = 128          # partition count / channel chunk
NB = 2048        # DMA big-tile pixels
NQ = 512         # logits quad pixels
NS = 128         # transpose subtile pixels
F32 = mybir.dt.float32
AF = mybir.ActivationFunctionType


def _body(ctx: ExitStack, tc: tile.TileContext, x, qk, m2, g0, ident, out,
          phases="ABC", sfx=""):
    nc = tc.nc

    def pool(name, **kw):
        return ctx.enter_context(tc.tile_pool(name=name + sfx, **kw))

    consts = pool("consts", bufs=1)
    qk0 = consts.tile([P, K], F32, tag="qk0")
    qk1 = consts.tile([P, K], F32, tag="qk1")
    m2t0 = consts.tile([P, C], F32, tag="m2t0")
    m2t1 = consts.tile([P, C], F32, tag="m2t1")
    g0_sb = consts.tile([P, 2], F32, tag="g0")
    id_sb = consts.tile([P, P], F32, tag="ident")
    s_acc = consts.tile([K, HW // NQ], F32, tag="s_acc")

    nc.sync.dma_start(qk0[:], qk[0:P, :])
    nc.sync.dma_start(qk1[:], qk[P : 2 * P, :])
    nc.sync.dma_start(m2t0[:], m2[0:P, :])
    nc.sync.dma_start(m2t1[:], m2[P : 2 * P, :])
    nc.sync.dma_start(g0_sb[:], g0[:, :])
    nc.sync.dma_start(id_sb[:], ident[:, :])

    xbig = pool("xbig", bufs=3)
    ps_small = pool("ps_small", bufs=3, space="PSUM")
    ps_xt = pool("ps_xt", bufs=2, space="PSUM")
    ps_y = pool("ps_y", bufs=1, space="PSUM")
    sb_xt = pool("sb_xt", bufs=3)
    sb_pt = pool("sb_pt", bufs=3)
    sb_p = pool("sb_p", bufs=3)
    xcpool = pool("xc", bufs=6)

    # ---------------- Phase A: stream x, build p, accumulate y and s ----------
    y_ps = ps_y.tile([K, C], F32, tag="y")
    n_big = HW // NB                 # 8
    n_quad = NB // NQ                # 4
    n_sub = NQ // NS                 # 4
    idx, last = 0, (HW // NS) - 1    # 128 y-matmuls
    for bt in range(n_big):
        xb0 = xbig.tile([P, NB], F32, tag="xb0")
        xb1 = xbig.tile([P, NB], F32, tag="xb1")
        nc.sync.dma_start(xb0[:], x[0:P, bt * NB : (bt + 1) * NB])
        nc.sync.dma_start(xb1[:], x[P : 2 * P, bt * NB : (bt + 1) * NB])
        for q in range(n_quad):
            t = bt * n_quad + q
            sl = slice(q * NQ, (q + 1) * NQ)
            l_ps = ps_small.tile([K, NQ], F32, tag="pssmall")
            nc.tensor.matmul(l_ps[:], qk0[:], xb0[:, sl], start=True, stop=False)
            nc.tensor.matmul(l_ps[:], qk1[:], xb1[:, sl], start=False, stop=True)
            p_sb = sb_p.tile([K, NQ], F32, tag="p")
            nc.scalar.activation(
                p_sb[:], l_ps[:], AF.Exp, accum_out=s_acc[:, t : t + 1]
            )
            pt_ps = ps_small.tile([P, n_sub * K], F32, tag="pssmall")
            for j in range(n_sub):
                nc.tensor.transpose(
                    pt_ps[:, j * K : (j + 1) * K],
                    p_sb[:, j * NS : (j + 1) * NS],
                    id_sb[0:K, 0:K],
                )
            pt_sb = sb_pt.tile([P, n_sub * K], F32, tag="ptsb")
            nc.vector.tensor_copy(pt_sb[:], pt_ps[:])

            xt_ps = ps_xt.tile([P, n_sub, C], F32, tag="xt")
            for j in range(n_sub):
                ss = slice(q * NQ + j * NS, q * NQ + (j + 1) * NS)
                nc.tensor.transpose(xt_ps[:, j, 0:P], xb0[:, ss], id_sb[:, :])
                nc.tensor.transpose(xt_ps[:, j, P : 2 * P], xb1[:, ss], id_sb[:, :])
            xt_sb = sb_xt.tile([P, n_sub, C], F32, tag="xtsb")
            nc.vector.tensor_copy(xt_sb[:], xt_ps[:])

            for j in range(n_sub):
                nc.tensor.matmul(
                    y_ps[:],
                    pt_sb[:, j * K : (j + 1) * K],
                    xt_sb[:, j, :],
                    start=(idx == 0),
                    stop=(idx == last),
                    skip_group_check=True,
                )
                idx += 1

    # ---------------- Phase B: softmax denominators -> xa -> gate -------------
    s_sb = consts.tile([K, 1], F32, tag="s_sb")
    nc.vector.reduce_sum(s_sb[:], s_acc[:], axis=mybir.AxisListType.X)
    r_sb = consts.tile([K, 1], F32, tag="r_sb")
    nc.vector.reciprocal(r_sb[:], s_sb[:])
    y_sb = consts.tile([K, C], F32, tag="y_sb")
    nc.vector.tensor_copy(y_sb[:], y_ps[:])

    xa_ps = ps_small.tile([1, C], F32, tag="pssmall")
    nc.tensor.matmul(xa_ps[:], r_sb[:], y_sb[:], start=True, stop=True)
    xa_sb = consts.tile([1, C], F32, tag="xa_sb")
    nc.vector.tensor_copy(xa_sb[:], xa_ps[:])

    xat_ps = ps_small.tile([P, 2], F32, tag="pssmall")
    for j in range(2):
        nc.tensor.transpose(
            xat_ps[:, j : j + 1], xa_sb[0:1, j * P : (j + 1) * P], id_sb[0:1, 0:1]
        )
    xat_sb = consts.tile([P, 2], F32, tag="xat_sb")
    nc.vector.tensor_copy(xat_sb[:], xat_ps[:])

    gate_ps = ps_small.tile([P, 2], F32, tag="pssmall")
    for cc in range(2):
        csl = slice(cc * P, (cc + 1) * P)
        nc.tensor.matmul(
            gate_ps[:, cc : cc + 1], m2t0[:, csl], xat_sb[:, 0:1],
            start=True, stop=False, skip_group_check=True,
        )
        nc.tensor.matmul(
            gate_ps[:, cc : cc + 1], m2t1[:, csl], xat_sb[:, 1:2],
            start=False, stop=True, skip_group_check=True,
        )
    gate_sb = consts.tile([P, 2], F32, tag="gate_sb")
    nc.vector.tensor_add(gate_sb[:], gate_ps[:], g0_sb[:])

    if "C" not in phases:
        nc.sync.dma_start(out[0:P, 0:2], gate_sb[:])
        return

    # ---------------- Phase C: out = x * gate ---------------------------------
    for cc in range(2):
        csl = slice(cc * P, (cc + 1) * P)
        for nt in range(HW // NB):
            xc = xcpool.tile([P, NB], F32, tag="xc")
            nsl = slice(nt * NB, (nt + 1) * NB)
            nc.sync.dma_start(xc[:], x[csl, nsl])
            nc.vector.tensor_scalar_mul(xc[:], xc[:], gate_sb[:, cc : cc + 1])
            nc.sync.dma_start(out[csl, nsl], xc[:])


def build_nc(repeats=1, body=None):
    body = body or _body
    nc = bacc.Bacc(
        "TRN2",
        target_bir_lowering=False,
        debug=False,
        enable_asserts=False,
        num_devices=B,
    )
    x = nc.dram_tensor("x", [C, HW], F32, kind="ExternalInput").ap()
    qk = nc.dram_tensor("qkT", [C, K], F32, kind="ExternalInput").ap()
    m2 = nc.dram_tensor("m2t", [C, C], F32, kind="ExternalInput").ap()
    g0 = nc.dram_tensor("g0", [P, 2], F32, kind="ExternalInput").ap()
    ident = nc.dram_tensor("ident", [P, P], F32, kind="ExternalInput").ap()
    out = nc.dram_tensor("out", [C, HW], F32, kind="ExternalOutput").ap()

    with tile.TileContext(nc) as tc:
        for r in range(repeats):
            with ExitStack() as ctx:
                body(ctx, tc, x, qk, m2, g0, ident, out, sfx=f"_{r}")
    nc.compile()
    return nc


_NC = None


def _get_nc():
    global _NC
    if _NC is None:
        _NC = build_nc()
    return _NC


def make_in_maps(x, query_embed, Wk, bk, Wv, bv, Wo, bo):
    x = np.asarray(x, dtype=np.float32)
    qe = np.asarray(query_embed, dtype=np.float64)
    Wk64 = np.asarray(Wk, dtype=np.float64)
    Wv64 = np.asarray(Wv, dtype=np.float64)
    Wo64 = np.asarray(Wo, dtype=np.float64)
    bv64 = np.asarray(bv, dtype=np.float64)
    bo64 = np.asarray(bo, dtype=np.float64)

    qkT = ((qe @ Wk64) / np.sqrt(float(D))).T.astype(np.float32).copy()
    m2t = (Wo64 @ Wv64).T.astype(np.float32).copy()
    g0 = (float(K) * (Wo64 @ bv64 + bo64)).astype(np.float32)
    g0c = np.ascontiguousarray(g0.reshape(2, P).T)
    ident = np.eye(P, dtype=np.float32)

    return [
        {
            "x": np.ascontiguousarray(x[b].reshape(C, HW)),
            "qkT": qkT,
            "m2t": m2t,
            "g0": g0c,
            "ident": ident,
        }
        for b in range(B)
    ]


def kernel(x, query_embed, Wk, bk, Wv, bv, Wo, bo, _trace=False, **kw):
    in_maps = make_in_maps(x, query_embed, Wk, bk, Wv, bv, Wo, bo)
    nc = _get_nc()
    res = run_bass_kernel_spmd(nc, in_maps, core_ids=list(range(B)), trace=_trace, **kw)
    out = np.stack(
        [res.results[b]["out"].reshape(C, 128, 128) for b in range(B)]
    ).astype(np.float32)
    if _trace:
        kernel.last_results = res
    return out


# revision 8
# speedup vs baseline: 18.9679x; 1.1385x over previous
"""Bass/Trainium2 kernel for nn_ClassQueryAttention.

Math (per batch b, x flattened to [C=256, N=16384]):
  logits[k,n] = (qe @ Wk) @ x / sqrt(D)          (per-k bias qe@bk cancels in softmax)
  p = exp(logits)  (no max-subtraction needed: logits ~ N(0,1))
  s_k = sum_n p[k,n];  r_k = 1/s_k
  y[k,c] = sum_n p[k,n] x[c,n]                   (flash-style, accumulated in PSUM)
  xa[c]  = sum_k r_k y[k,c]
  gate   = (Wo@Wv) @ xa + K*(Wo@bv + bo)
  out[c,n] = x[c,n] * gate[c]

Sharding: data-parallel over batch B=8, one batch per NeuronCore, no collectives.
Per-core HBM traffic: 2 reads of x (16 MiB each) + 1 write (16 MiB) = 48 MiB.
"""

import sys
from contextlib import ExitStack

import numpy as np

sys.path.insert(0, "/opt/trn_rl_repo")

import concourse.bass as bass  # noqa: E402
import concourse.tile as tile  # noqa: E402
from concourse import bacc, mybir  # noqa: E402
from concourse.bass_utils import run_bass_kernel_spmd  # noqa: E402

B, C, HW = 8, 256, 128 * 128
K, D = 21, 256
P = 128          # partition count / channel chunk
NB = 2048        # DMA big-tile pixels
NQ = 512         # logits quad pixels
NS = 128         # transpose subtile pixels
F32 = mybir.dt.float32
AF = mybir.ActivationFunctionType


def _body(ctx: ExitStack, tc: tile.TileContext, x, qk, m2, g0, ident, out,
          phases="ABC", sfx=""):
    nc = tc.nc

    def pool(name, **kw):
        return ctx.enter_context(tc.tile_pool(name=name + sfx, **kw))

    consts = pool("consts", bufs=1)
    qk0 = consts.tile([P, K], F32, tag="qk0")
    qk1 = consts.tile([P, K], F32, tag="qk1")
    m2t0 = consts.tile([P, C], F32, tag="m2t0")
    m2t1 = consts.tile([P, C], F32, tag="m2t1")
    g0_sb = consts.tile([P, 2], F32, tag="g0")
    id_sb = consts.tile([P, P], F32, tag="ident")
    s_acc = consts.tile([K, HW // NQ], F32, tag="s_acc")

    nc.sync.dma_start(qk0[:], qk[0:P, :])
    nc.sync.dma_start(qk1[:], qk[P : 2 * P, :])
    nc.sync.dma_start(m2t0[:], m2[0:P, :])
    nc.sync.dma_start(m2t1[:], m2[P : 2 * P, :])
    nc.sync.dma_start(g0_sb[:], g0[:, :])
    nc.sync.dma_start(id_sb[:], ident[:, :])

    xbig = pool("xbig", bufs=1)
    ps_small = pool("ps_small", bufs=3, space="PSUM")
    ps_xt = pool("ps_xt", bufs=2, space="PSUM")
    ps_y = pool("ps_y", bufs=1, space="PSUM")
    sb_xt = pool("sb_xt", bufs=3)
    sb_pt = pool("sb_pt", bufs=3)
    sb_p = pool("sb_p", bufs=3)

    # ---------------- Phase A: stream x, build p, accumulate y and s ----------
    # x stays resident in SBUF (16 tiles x 8 KiB/partition) -> phase C needs no
    # second HBM read.
    y_ps = ps_y.tile([K, C], F32, tag="y")
    n_big = HW // NB                 # 8
    n_quad = NB // NQ                # 4
    n_sub = NQ // NS                 # 4
    idx, last = 0, (HW // NS) - 1    # 128 y-matmuls
    xres = {}
    for bt in range(n_big):
        xb0 = xbig.tile([P, NB], F32, tag=f"xb0_{bt}")
        xb1 = xbig.tile([P, NB], F32, tag=f"xb1_{bt}")
        xres[0, bt], xres[1, bt] = xb0, xb1
        nc.sync.dma_start(xb0[:], x[0:P, bt * NB : (bt + 1) * NB])
        nc.sync.dma_start(xb1[:], x[P : 2 * P, bt * NB : (bt + 1) * NB])
        for q in range(n_quad):
            t = bt * n_quad + q
            sl = slice(q * NQ, (q + 1) * NQ)
            l_ps = ps_small.tile([K, NQ], F32, tag="pssmall")
            nc.tensor.matmul(l_ps[:], qk0[:], xb0[:, sl], start=True, stop=False)
            nc.tensor.matmul(l_ps[:], qk1[:], xb1[:, sl], start=False, stop=True)
            p_sb = sb_p.tile([K, NQ], F32, tag="p")
            nc.scalar.activation(
                p_sb[:], l_ps[:], AF.Exp, accum_out=s_acc[:, t : t + 1]
            )
            pt_ps = ps_small.tile([P, n_sub * K], F32, tag="pssmall")
            for j in range(n_sub):
                nc.tensor.transpose(
                    pt_ps[:, j * K : (j + 1) * K],
                    p_sb[:, j * NS : (j + 1) * NS],
                    id_sb[0:K, 0:K],
                )
            pt_sb = sb_pt.tile([P, n_sub * K], F32, tag="ptsb")
            nc.vector.tensor_copy(pt_sb[:], pt_ps[:])

            xt_ps = ps_xt.tile([P, n_sub, C], F32, tag="xt")
            for j in range(n_sub):
                ss = slice(q * NQ + j * NS, q * NQ + (j + 1) * NS)
                nc.tensor.transpose(xt_ps[:, j, 0:P], xb0[:, ss], id_sb[:, :])
                nc.tensor.transpose(xt_ps[:, j, P : 2 * P], xb1[:, ss], id_sb[:, :])
            xt_sb = sb_xt.tile([P, n_sub, C], F32, tag="xtsb")
            nc.vector.tensor_copy(xt_sb[:], xt_ps[:])

            for j in range(n_sub):
                nc.tensor.matmul(
                    y_ps[:],
                    pt_sb[:, j * K : (j + 1) * K],
                    xt_sb[:, j, :],
                    start=(idx == 0),
                    stop=(idx == last),
                    skip_group_check=True,
                )
                idx += 1

    # ---------------- Phase B: softmax denominators -> xa -> gate -------------
    s_sb = consts.tile([K, 1], F32, tag="s_sb")
    nc.vector.reduce_sum(s_sb[:], s_acc[:], axis=mybir.AxisListType.X)
    r_sb = consts.tile([K, 1], F32, tag="r_sb")
    nc.vector.reciprocal(r_sb[:], s_sb[:])
    y_sb = consts.tile([K, C], F32, tag="y_sb")
    nc.vector.tensor_copy(y_sb[:], y_ps[:])

    xa_ps = ps_small.tile([1, C], F32, tag="pssmall")
    nc.tensor.matmul(xa_ps[:], r_sb[:], y_sb[:], start=True, stop=True)
    xa_sb = consts.tile([1, C], F32, tag="xa_sb")
    nc.vector.tensor_copy(xa_sb[:], xa_ps[:])

    xat_ps = ps_small.tile([P, 2], F32, tag="pssmall")
    for j in range(2):
        nc.tensor.transpose(
            xat_ps[:, j : j + 1], xa_sb[0:1, j * P : (j + 1) * P], id_sb[0:1, 0:1]
        )
    xat_sb = consts.tile([P, 2], F32, tag="xat_sb")
    nc.vector.tensor_copy(xat_sb[:], xat_ps[:])

    gate_ps = ps_small.tile([P, 2], F32, tag="pssmall")
    for cc in range(2):
        csl = slice(cc * P, (cc + 1) * P)
        nc.tensor.matmul(
            gate_ps[:, cc : cc + 1], m2t0[:, csl], xat_sb[:, 0:1],
            start=True, stop=False, skip_group_check=True,
        )
        nc.tensor.matmul(
            gate_ps[:, cc : cc + 1], m2t1[:, csl], xat_sb[:, 1:2],
            start=False, stop=True, skip_group_check=True,
        )
    gate_sb = consts.tile([P, 2], F32, tag="gate_sb")
    nc.vector.tensor_add(gate_sb[:], gate_ps[:], g0_sb[:])

    if "C" not in phases:
        nc.sync.dma_start(out[0:P, 0:2], gate_sb[:])
        return

    # ---------------- Phase C: out = x * gate (x already in SBUF) -------------
    for cc in range(2):
        csl = slice(cc * P, (cc + 1) * P)
        for nt in range(HW // NB):
            xc = xres[cc, nt]
            nsl = slice(nt * NB, (nt + 1) * NB)
            nc.vector.tensor_scalar_mul(xc[:], xc[:], gate_sb[:, cc : cc + 1])
            nc.sync.dma_start(out[csl, nsl], xc[:])


def build_nc(repeats=1, body=None):
    body = body or _body
    nc = bacc.Bacc(
        "TRN2",
        target_bir_lowering=False,
        debug=False,
        enable_asserts=False,
        num_devices=B,
    )
    x = nc.dram_tensor("x", [C, HW], F32, kind="ExternalInput").ap()
    qk = nc.dram_tensor("qkT", [C, K], F32, kind="ExternalInput").ap()
    m2 = nc.dram_tensor("m2t", [C, C], F32, kind="ExternalInput").ap()
    g0 = nc.dram_tensor("g0", [P, 2], F32, kind="ExternalInput").ap()
    ident = nc.dram_tensor("ident", [P, P], F32, kind="ExternalInput").ap()
    out = nc.dram_tensor("out", [C, HW], F32, kind="ExternalOutput").ap()

    with tile.TileContext(nc) as tc:
        for r in range(repeats):
            with ExitStack() as ctx:
                body(ctx, tc, x, qk, m2, g0, ident, out, sfx=f"_{r}")
    nc.compile()
    return nc


_NC = None


def _get_nc():
    global _NC
    if _NC is None:
        _NC = build_nc()
    return _NC


def make_in_maps(x, query_embed, Wk, bk, Wv, bv, Wo, bo):
    x = np.asarray(x, dtype=np.float32)
    qe = np.asarray(query_embed, dtype=np.float64)
    Wk64 = np.asarray(Wk, dtype=np.float64)
    Wv64 = np.asarray(Wv, dtype=np.float64)
    Wo64 = np.asarray(Wo, dtype=np.float64)
    bv64 = np.asarray(bv, dtype=np.float64)
    bo64 = np.asarray(bo, dtype=np.float64)

    qkT = ((qe @ Wk64) / np.sqrt(float(D))).T.astype(np.float32).copy()
    m2t = (Wo64 @ Wv64).T.astype(np.float32).copy()
    g0 = (float(K) * (Wo64 @ bv64 + bo64)).astype(np.float32)
    g0c = np.ascontiguousarray(g0.reshape(2, P).T)
    ident = np.eye(P, dtype=np.float32)

    return [
        {
            "x": np.ascontiguousarray(x[b].reshape(C, HW)),
            "qkT": qkT,
            "m2t": m2t,
            "g0": g0c,
            "ident": ident,
        }
        for b in range(B)
    ]


def kernel(x, query_embed, Wk, bk, Wv, bv, Wo, bo, _trace=False, **kw):
    in_maps = make_in_maps(x, query_embed, Wk, bk, Wv, bv, Wo, bo)
    nc = _get_nc()
    res = run_bass_kernel_spmd(nc, in_maps, core_ids=list(range(B)), trace=_trace, **kw)
    out = np.stack(
        [res.results[b]["out"].reshape(C, 128, 128) for b in range(B)]
    ).astype(np.float32)
    if _trace:
        kernel.last_results = res
    return out


# revision 11
# speedup vs baseline: 26.5628x; 1.4004x over previous
"""Bass/Trainium2 kernel for nn_ClassQueryAttention.

Math (per batch b, x flattened to [C=256, N=16384]):
  logits[k,n] = (qe @ Wk) @ x / sqrt(D)          (per-k bias qe@bk cancels in softmax)
  p = exp(logits)  (no max-subtraction needed: logits ~ N(0,1))
  s_k = sum_n p[k,n];  r_k = 1/s_k
  y[k,c] = sum_n p[k,n] x[c,n]                   (flash-style, accumulated in PSUM)
  xa[c]  = sum_k r_k y[k,c]
  gate   = (Wo@Wv) @ xa + K*(Wo@bv + bo)
  out[c,n] = x[c,n] * gate[c]

Sharding: data-parallel over batch B=8, one batch per NeuronCore, no collectives.
Per-core HBM traffic: 2 reads of x (16 MiB each) + 1 write (16 MiB) = 48 MiB.
"""

import sys
from contextlib import ExitStack

import numpy as np

sys.path.insert(0, "/opt/trn_rl_repo")

import concourse.bass as bass  # noqa: E402
import concourse.tile as tile  # noqa: E402
from concourse import bacc, mybir  # noqa: E402
from concourse.bass_utils import run_bass_kernel_spmd  # noqa: E402

B, C, HW = 8, 256, 128 * 128
K, D = 21, 256
P = 128          # partition count / channel chunk
NB = 2048        # DMA big-tile pixels
NQ = 512         # logits quad pixels
NS = 128         # transpose subtile pixels
F32 = mybir.dt.float32
AF = mybir.ActivationFunctionType


def _body(ctx: ExitStack, tc: tile.TileContext, x, qk, m2, g0, ident, out,
          phases="ABC", sfx="", stages="LXPY"):
    nc = tc.nc

    def pool(name, **kw):
        return ctx.enter_context(tc.tile_pool(name=name + sfx, **kw))

    consts = pool("consts", bufs=1)
    qk0 = consts.tile([P, K], F32, tag="qk0")
    qk1 = consts.tile([P, K], F32, tag="qk1")
    m2t0 = consts.tile([P, C], F32, tag="m2t0")
    m2t1 = consts.tile([P, C], F32, tag="m2t1")
    g0_sb = consts.tile([P, 2], F32, tag="g0")
    id_sb = consts.tile([P, P], F32, tag="ident")
    s_acc = consts.tile([K, HW // NQ], F32, tag="s_acc")

    nc.sync.dma_start(qk0[:], qk[0:P, :])
    nc.sync.dma_start(qk1[:], qk[P : 2 * P, :])
    nc.sync.dma_start(m2t0[:], m2[0:P, :])
    nc.sync.dma_start(m2t1[:], m2[P : 2 * P, :])
    nc.sync.dma_start(g0_sb[:], g0[:, :])
    nc.sync.dma_start(id_sb[:], ident[:, :])

    xbig = pool("xbig", bufs=1)
    ps_small = pool("ps_small", bufs=3, space="PSUM")
    ps_xt = pool("ps_xt", bufs=2, space="PSUM")
    ps_y = pool("ps_y", bufs=1, space="PSUM")
    sb_xt = pool("sb_xt", bufs=3)
    sb_pt = pool("sb_pt", bufs=3)
    sb_p = pool("sb_p", bufs=3)

    # ---------------- Phase A: stream x, build p, accumulate y and s ----------
    # x stays resident in SBUF (16 tiles x 8 KiB/partition) -> phase C needs no
    # second HBM read.
    y_ps = ps_y.tile([P, C], F32, tag="y")
    n_big = HW // NB                 # 8
    n_quad = NB // NQ                # 4
    n_sub = NQ // NS                 # 4
    idx, last = 0, (HW // NS) - 1    # 128 y-matmuls
    xres = {}
    for bt in range(n_big):
        xb0 = xbig.tile([P, NB], F32, tag=f"xb0_{bt}")
        xb1 = xbig.tile([P, NB], F32, tag=f"xb1_{bt}")
        xres[0, bt], xres[1, bt] = xb0, xb1
        nc.sync.dma_start(xb0[:], x[0:P, bt * NB : (bt + 1) * NB])
        nc.sync.dma_start(xb1[:], x[P : 2 * P, bt * NB : (bt + 1) * NB])
        for q in range(n_quad):
            t = bt * n_quad + q
            sl = slice(q * NQ, (q + 1) * NQ)
            pt_sb = None
            if "L" in stages:
                l_ps = ps_small.tile([K, NQ], F32, tag="pssmall")
                nc.tensor.matmul(l_ps[:], qk0[:], xb0[:, sl], start=True, stop=False)
                nc.tensor.matmul(l_ps[:], qk1[:], xb1[:, sl], start=False, stop=True)
                p_sb = sb_p.tile([K, NQ], F32, tag="p")
                nc.scalar.activation(
                    p_sb[:], l_ps[:], AF.Exp, accum_out=s_acc[:, t : t + 1]
                )
                if "P" in stages:
                    pt_ps = ps_small.tile([P, n_sub * K], F32, tag="pssmall")
                    for j in range(n_sub):
                        nc.tensor.transpose(
                            pt_ps[:, j * K : (j + 1) * K],
                            p_sb[:, j * NS : (j + 1) * NS],
                            id_sb[0:K, 0:K],
                        )
                    pt_sb = sb_pt.tile([P, n_sub * K], F32, tag="ptsb")
                    nc.vector.tensor_copy(pt_sb[:], pt_ps[:])

            xt_sb = None
            if "X" in stages:
                xt_ps = ps_xt.tile([P, n_sub, C], F32, tag="xt")
                for j in range(n_sub):
                    ss = slice(q * NQ + j * NS, q * NQ + (j + 1) * NS)
                    nc.tensor.transpose(xt_ps[:, j, 0:P], xb0[:, ss], id_sb[:, :])
                    nc.tensor.transpose(xt_ps[:, j, P : 2 * P], xb1[:, ss], id_sb[:, :])
                xt_sb = sb_xt.tile([P, n_sub, C], F32, tag="xtsb")
                # alternate copy engine to balance DVE vs ACT load
                if t % 2 == 0:
                    nc.vector.tensor_copy(xt_sb[:], xt_ps[:])
                else:
                    nc.scalar.copy(xt_sb[:], xt_ps[:])

            if "Y" in stages:
                # 4 concurrent matmuls in distinct PE column groups (M=21<=32)
                for j in range(n_sub):
                    nc.tensor.matmul(
                        y_ps[32 * j : 32 * j + K, :],
                        pt_sb[:, j * K : (j + 1) * K],
                        xt_sb[:, j, :],
                        start=(t == 0),
                        stop=(t == HW // NQ - 1),
                        skip_group_check=True,
                        tile_position=(0, 32 * j),
                    )
                    idx += 1

    # ---------------- Phase B: softmax denominators -> xa -> gate -------------
    if "Y" not in stages:
        gate_sb = consts.tile([P, 2], F32, tag="gate_sb")
        nc.vector.tensor_copy(gate_sb[:], g0_sb[:])
        if "C" in phases:
            for cc in range(2):
                csl = slice(cc * P, (cc + 1) * P)
                for nt in range(HW // NB):
                    xc = xres[cc, nt]
                    nsl = slice(nt * NB, (nt + 1) * NB)
                    nc.vector.tensor_scalar_mul(xc[:], xc[:], gate_sb[:, cc : cc + 1])
                    nc.sync.dma_start(out[csl, nsl], xc[:])
        else:
            nc.sync.dma_start(out[0:P, 0:2], gate_sb[:])
        return
    s_sb = consts.tile([K, 1], F32, tag="s_sb")
    nc.vector.reduce_sum(s_sb[:], s_acc[:], axis=mybir.AxisListType.X)
    r_sb = consts.tile([K, 1], F32, tag="r_sb")
    nc.vector.reciprocal(r_sb[:], s_sb[:])
    # y lives in 4 col-group blocks at partitions 32j..32j+20. Instead of
    # folding them, replicate r into the same blocks (zeros elsewhere) and let
    # the xa matmul contract all 128 partitions in one shot.
    yf_sb = consts.tile([P, C], F32, tag="yf_sb")
    nc.vector.memset(yf_sb[:], 0.0)
    for j in range(4):
        nc.vector.tensor_copy(yf_sb[32 * j : 32 * j + K, :], y_ps[32 * j : 32 * j + K, :])
    r4_sb = consts.tile([P, 1], F32, tag="r4_sb")
    nc.vector.memset(r4_sb[:], 0.0)
    r4v = r4_sb.rearrange("(a b) c -> a b c", b=32)
    for j in range(4):
        nc.sync.dma_start(r4v[j, 0:K, :], r_sb[:])

    xa_ps = ps_small.tile([1, C], F32, tag="pssmall")
    nc.tensor.matmul(xa_ps[:], r4_sb[:], yf_sb[:], start=True, stop=True)
    xa_sb = consts.tile([1, C], F32, tag="xa_sb")
    nc.vector.tensor_copy(xa_sb[:], xa_ps[:])

    xat_ps = ps_small.tile([P, 2], F32, tag="pssmall")
    for j in range(2):
        nc.tensor.transpose(
            xat_ps[:, j : j + 1], xa_sb[0:1, j * P : (j + 1) * P], id_sb[0:1, 0:1]
        )
    xat_sb = consts.tile([P, 2], F32, tag="xat_sb")
    nc.vector.tensor_copy(xat_sb[:], xat_ps[:])

    gate_ps = ps_small.tile([P, 2], F32, tag="pssmall")
    for cc in range(2):
        csl = slice(cc * P, (cc + 1) * P)
        nc.tensor.matmul(
            gate_ps[:, cc : cc + 1], m2t0[:, csl], xat_sb[:, 0:1],
            start=True, stop=False, skip_group_check=True,
        )
        nc.tensor.matmul(
            gate_ps[:, cc : cc + 1], m2t1[:, csl], xat_sb[:, 1:2],
            start=False, stop=True, skip_group_check=True,
        )
    gate_sb = consts.tile([P, 2], F32, tag="gate_sb")
    nc.vector.tensor_add(gate_sb[:], gate_ps[:], g0_sb[:])

    if "C" not in phases:
        nc.sync.dma_start(out[0:P, 0:2], gate_sb[:])
        return

    # ---------------- Phase C: out = x * gate (x already in SBUF) -------------
    for cc in range(2):
        csl = slice(cc * P, (cc + 1) * P)
        for nt in range(HW // NB):
            xc = xres[cc, nt]
            nsl = slice(nt * NB, (nt + 1) * NB)
            if nt % 2 == 0:
                nc.vector.tensor_scalar_mul(xc[:], xc[:], gate_sb[:, cc : cc + 1])
            else:
                nc.scalar.mul(xc[:], xc[:], gate_sb[:, cc : cc + 1])
            nc.sync.dma_start(out[csl, nsl], xc[:])


def build_nc(repeats=1, body=None):
    body = body or _body
    nc = bacc.Bacc(
        "TRN2",
        target_bir_lowering=False,
        debug=False,
        enable_asserts=False,
        num_devices=B,
    )
    x = nc.dram_tensor("x", [C, HW], F32, kind="ExternalInput").ap()
    qk = nc.dram_tensor("qkT", [C, K], F32, kind="ExternalInput").ap()
    m2 = nc.dram_tensor("m2t", [C, C], F32, kind="ExternalInput").ap()
    g0 = nc.dram_tensor("g0", [P, 2], F32, kind="ExternalInput").ap()
    ident = nc.dram_tensor("ident", [P, P], F32, kind="ExternalInput").ap()
    out = nc.dram_tensor("out", [C, HW], F32, kind="ExternalOutput").ap()

    with tile.TileContext(nc) as tc:
        for r in range(repeats):
            with ExitStack() as ctx:
                body(ctx, tc, x, qk, m2, g0, ident, out, sfx=f"_{r}")
    nc.compile()
    return nc


_NC = None


def _get_nc():
    global _NC
    if _NC is None:
        _NC = build_nc()
    return _NC


def make_in_maps(x, query_embed, Wk, bk, Wv, bv, Wo, bo):
    x = np.asarray(x, dtype=np.float32)
    qe = np.asarray(query_embed, dtype=np.float64)
    Wk64 = np.asarray(Wk, dtype=np.float64)
    Wv64 = np.asarray(Wv, dtype=np.float64)
    Wo64 = np.asarray(Wo, dtype=np.float64)
    bv64 = np.asarray(bv, dtype=np.float64)
    bo64 = np.asarray(bo, dtype=np.float64)

    qkT = ((qe @ Wk64) / np.sqrt(float(D))).T.astype(np.float32).copy()
    m2t = (Wo64 @ Wv64).T.astype(np.float32).copy()
    g0 = (float(K) * (Wo64 @ bv64 + bo64)).astype(np.float32)
    g0c = np.ascontiguousarray(g0.reshape(2, P).T)
    ident = np.eye(P, dtype=np.float32)

    return [
        {
            "x": np.ascontiguousarray(x[b].reshape(C, HW)),
            "qkT": qkT,
            "m2t": m2t,
            "g0": g0c,
            "ident": ident,
        }
        for b in range(B)
    ]


def kernel(x, query_embed, Wk, bk, Wv, bv, Wo, bo, _trace=False, **kw):
    in_maps = make_in_maps(x, query_embed, Wk, bk, Wv, bv, Wo, bo)
    nc = _get_nc()
    res = run_bass_kernel_spmd(nc, in_maps, core_ids=list(range(B)), trace=_trace, **kw)
    out = np.stack(
        [res.results[b]["out"].reshape(C, 128, 128) for b in range(B)]
    ).astype(np.float32)
    if _trace:
        kernel.last_results = res
    return out


# revision 12
# speedup vs baseline: 29.9818x; 1.1287x over previous
"""Bass/Trainium2 kernel for nn_ClassQueryAttention.

Math (per batch b, x flattened to [C=256, N=16384]):
  logits[k,n] = (qe @ Wk) @ x / sqrt(D)          (per-k bias qe@bk cancels in softmax)
  p = exp(logits)  (no max-subtraction needed: logits ~ N(0,1))
  s_k = sum_n p[k,n];  r_k = 1/s_k
  y[k,c] = sum_n p[k,n] x[c,n]                   (flash-style, accumulated in PSUM)
  xa[c]  = sum_k r_k y[k,c]
  gate   = (Wo@Wv) @ xa + K*(Wo@bv + bo)
  out[c,n] = x[c,n] * gate[c]

Sharding: data-parallel over batch B=8, one batch per NeuronCore, no collectives.
Per-core HBM traffic: 2 reads of x (16 MiB each) + 1 write (16 MiB) = 48 MiB.
"""

import sys
from contextlib import ExitStack

import numpy as np

sys.path.insert(0, "/opt/trn_rl_repo")

import concourse.bass as bass  # noqa: E402
import concourse.tile as tile  # noqa: E402
from concourse import bacc, mybir  # noqa: E402
from concourse.bass_utils import run_bass_kernel_spmd  # noqa: E402

B, C, HW = 8, 256, 128 * 128
K, D = 21, 256
P = 128          # partition count / channel chunk
NB = 2048        # DMA big-tile pixels
NQ = 512         # logits quad pixels
NS = 128         # transpose subtile pixels
F32 = mybir.dt.float32
AF = mybir.ActivationFunctionType


def _body(ctx: ExitStack, tc: tile.TileContext, x, qk, m2, g0, ident, out,
          phases="ABC", sfx="", stages="LXPY"):
    nc = tc.nc

    def pool(name, **kw):
        return ctx.enter_context(tc.tile_pool(name=name + sfx, **kw))

    consts = pool("consts", bufs=1)
    qk0 = consts.tile([P, K], F32, tag="qk0")
    qk1 = consts.tile([P, K], F32, tag="qk1")
    m2t0 = consts.tile([P, C], F32, tag="m2t0")
    m2t1 = consts.tile([P, C], F32, tag="m2t1")
    g0_sb = consts.tile([P, 2], F32, tag="g0")
    id_sb = consts.tile([P, P], F32, tag="ident")
    s_acc = consts.tile([K, HW // NQ], F32, tag="s_acc")

    nc.sync.dma_start(qk0[:], qk[0:P, :])
    nc.sync.dma_start(qk1[:], qk[P : 2 * P, :])
    nc.sync.dma_start(m2t0[:], m2[0:P, :])
    nc.sync.dma_start(m2t1[:], m2[P : 2 * P, :])
    nc.sync.dma_start(g0_sb[:], g0[:, :])
    nc.sync.dma_start(id_sb[:], ident[:, :])

    xbig = pool("xbig", bufs=1)
    ps_l = pool("ps_l", bufs=2, space="PSUM")      # logits [21,512]: 2 banks
    ps_pt = pool("ps_pt", bufs=2, space="PSUM")    # pT [128,84]: 2 banks
    ps_xt = pool("ps_xt", bufs=3, space="PSUM")    # xT halves [128,512]: 3 banks
    ps_y = pool("ps_y", bufs=1, space="PSUM")      # y accum: 1 bank
    sb_xt = pool("sb_xt", bufs=4)
    sb_pt = pool("sb_pt", bufs=4)
    sb_p = pool("sb_p", bufs=4)

    # ---------------- Phase A: stream x, build p, accumulate y and s ----------
    # x stays resident in SBUF (16 tiles x 8 KiB/partition) -> phase C needs no
    # second HBM read.
    y_ps = ps_y.tile([P, C], F32, tag="y")
    n_big = HW // NB                 # 8
    n_quad = NB // NQ                # 4
    n_sub = NQ // NS                 # 4
    idx, last = 0, (HW // NS) - 1    # 128 y-matmuls
    xres = {}
    for bt in range(n_big):
        xb0 = xbig.tile([P, NB], F32, tag=f"xb0_{bt}")
        xb1 = xbig.tile([P, NB], F32, tag=f"xb1_{bt}")
        xres[0, bt], xres[1, bt] = xb0, xb1
        nc.sync.dma_start(xb0[:], x[0:P, bt * NB : (bt + 1) * NB])
        nc.sync.dma_start(xb1[:], x[P : 2 * P, bt * NB : (bt + 1) * NB])
        for q in range(n_quad):
            t = bt * n_quad + q
            sl = slice(q * NQ, (q + 1) * NQ)
            pt_sb = None
            if "L" in stages:
                l_ps = ps_l.tile([K, NQ], F32, tag="l")
                nc.tensor.matmul(l_ps[:], qk0[:], xb0[:, sl], start=True, stop=False)
                nc.tensor.matmul(l_ps[:], qk1[:], xb1[:, sl], start=False, stop=True)
                p_sb = sb_p.tile([K, NQ], F32, tag="p")
                nc.scalar.activation(
                    p_sb[:], l_ps[:], AF.Exp, accum_out=s_acc[:, t : t + 1]
                )
                if "P" in stages:
                    pt_ps = ps_pt.tile([P, n_sub * K], F32, tag="pt")
                    for j in range(n_sub):
                        nc.tensor.transpose(
                            pt_ps[:, j * K : (j + 1) * K],
                            p_sb[:, j * NS : (j + 1) * NS],
                            id_sb[0:K, 0:K],
                        )
                    pt_sb = sb_pt.tile([P, n_sub * K], F32, tag="ptsb")
                    nc.vector.tensor_copy(pt_sb[:], pt_ps[:])

            xt_sb = None
            if "X" in stages:
                xt_sb = sb_xt.tile([P, n_sub, C], F32, tag="xtsb")
                for h in range(2):
                    xt_ps = ps_xt.tile([P, 2, C], F32, tag="xt")
                    for jj in range(2):
                        j = 2 * h + jj
                        ss = slice(q * NQ + j * NS, q * NQ + (j + 1) * NS)
                        nc.tensor.transpose(xt_ps[:, jj, 0:P], xb0[:, ss], id_sb[:, :])
                        nc.tensor.transpose(xt_ps[:, jj, P : 2 * P], xb1[:, ss], id_sb[:, :])
                    # alternate copy engine to balance DVE vs ACT load
                    if (2 * t + h) % 2 == 0:
                        nc.vector.tensor_copy(xt_sb[:, 2 * h : 2 * h + 2, :], xt_ps[:])
                    else:
                        nc.scalar.copy(xt_sb[:, 2 * h : 2 * h + 2, :], xt_ps[:])

            if "Y" in stages:
                # 4 concurrent matmuls in distinct PE column groups (M=21<=32)
                for j in range(n_sub):
                    nc.tensor.matmul(
                        y_ps[32 * j : 32 * j + K, :],
                        pt_sb[:, j * K : (j + 1) * K],
                        xt_sb[:, j, :],
                        start=(t == 0),
                        stop=(t == HW // NQ - 1),
                        skip_group_check=True,
                        tile_position=(0, 32 * j),
                    )
                    idx += 1

    # ---------------- Phase B: softmax denominators -> xa -> gate -------------
    if "Y" not in stages:
        gate_sb = consts.tile([P, 2], F32, tag="gate_sb")
        nc.vector.tensor_copy(gate_sb[:], g0_sb[:])
        if "C" in phases:
            for cc in range(2):
                csl = slice(cc * P, (cc + 1) * P)
                for nt in range(HW // NB):
                    xc = xres[cc, nt]
                    nsl = slice(nt * NB, (nt + 1) * NB)
                    nc.vector.tensor_scalar_mul(xc[:], xc[:], gate_sb[:, cc : cc + 1])
                    nc.sync.dma_start(out[csl, nsl], xc[:])
        else:
            nc.sync.dma_start(out[0:P, 0:2], gate_sb[:])
        return
    s_sb = consts.tile([K, 1], F32, tag="s_sb")
    nc.vector.reduce_sum(s_sb[:], s_acc[:], axis=mybir.AxisListType.X)
    r_sb = consts.tile([K, 1], F32, tag="r_sb")
    nc.vector.reciprocal(r_sb[:], s_sb[:])
    # y lives in 4 col-group blocks at partitions 32j..32j+20. Instead of
    # folding them, replicate r into the same blocks (zeros elsewhere) and let
    # the xa matmul contract all 128 partitions in one shot.
    yf_sb = consts.tile([P, C], F32, tag="yf_sb")
    nc.vector.memset(yf_sb[:], 0.0)
    for j in range(4):
        nc.vector.tensor_copy(yf_sb[32 * j : 32 * j + K, :], y_ps[32 * j : 32 * j + K, :])
    r4_sb = consts.tile([P, 1], F32, tag="r4_sb")
    nc.vector.memset(r4_sb[:], 0.0)
    r4v = r4_sb.rearrange("(a b) c -> a b c", b=32)
    for j in range(4):
        nc.sync.dma_start(r4v[j, 0:K, :], r_sb[:])

    xa_ps = ps_l.tile([1, C], F32, tag="l")
    nc.tensor.matmul(xa_ps[:], r4_sb[:], yf_sb[:], start=True, stop=True)
    xa_sb = consts.tile([1, C], F32, tag="xa_sb")
    nc.vector.tensor_copy(xa_sb[:], xa_ps[:])

    xat_ps = ps_pt.tile([P, 2], F32, tag="pt")
    for j in range(2):
        nc.tensor.transpose(
            xat_ps[:, j : j + 1], xa_sb[0:1, j * P : (j + 1) * P], id_sb[0:1, 0:1]
        )
    xat_sb = consts.tile([P, 2], F32, tag="xat_sb")
    nc.vector.tensor_copy(xat_sb[:], xat_ps[:])

    gate_ps = ps_l.tile([P, 2], F32, tag="l")
    for cc in range(2):
        csl = slice(cc * P, (cc + 1) * P)
        nc.tensor.matmul(
            gate_ps[:, cc : cc + 1], m2t0[:, csl], xat_sb[:, 0:1],
            start=True, stop=False, skip_group_check=True,
        )
        nc.tensor.matmul(
            gate_ps[:, cc : cc + 1], m2t1[:, csl], xat_sb[:, 1:2],
            start=False, stop=True, skip_group_check=True,
        )
    gate_sb = consts.tile([P, 2], F32, tag="gate_sb")
    nc.vector.tensor_add(gate_sb[:], gate_ps[:], g0_sb[:])

    if "C" not in phases:
        nc.sync.dma_start(out[0:P, 0:2], gate_sb[:])
        return

    # ---------------- Phase C: out = x * gate (x already in SBUF) -------------
    for cc in range(2):
        csl = slice(cc * P, (cc + 1) * P)
        for nt in range(HW // NB):
            xc = xres[cc, nt]
            nsl = slice(nt * NB, (nt + 1) * NB)
            if nt % 2 == 0:
                nc.vector.tensor_scalar_mul(xc[:], xc[:], gate_sb[:, cc : cc + 1])
            else:
                nc.scalar.mul(xc[:], xc[:], gate_sb[:, cc : cc + 1])
            nc.sync.dma_start(out[csl, nsl], xc[:])


def build_nc(repeats=1, body=None):
    body = body or _body
    nc = bacc.Bacc(
        "TRN2",
        target_bir_lowering=False,
        debug=False,
        enable_asserts=False,
        num_devices=B,
    )
    x = nc.dram_tensor("x", [C, HW], F32, kind="ExternalInput").ap()
    qk = nc.dram_tensor("qkT", [C, K], F32, kind="ExternalInput").ap()
    m2 = nc.dram_tensor("m2t", [C, C], F32, kind="ExternalInput").ap()
    g0 = nc.dram_tensor("g0", [P, 2], F32, kind="ExternalInput").ap()
    ident = nc.dram_tensor("ident", [P, P], F32, kind="ExternalInput").ap()
    out = nc.dram_tensor("out", [C, HW], F32, kind="ExternalOutput").ap()

    with tile.TileContext(nc) as tc:
        for r in range(repeats):
            with ExitStack() as ctx:
                body(ctx, tc, x, qk, m2, g0, ident, out, sfx=f"_{r}")
    nc.compile()
    return nc


_NC = None


def _get_nc():
    global _NC
    if _NC is None:
        _NC = build_nc()
    return _NC


def make_in_maps(x, query_embed, Wk, bk, Wv, bv, Wo, bo):
    x = np.asarray(x, dtype=np.float32)
    qe = np.asarray(query_embed, dtype=np.float64)
    Wk64 = np.asarray(Wk, dtype=np.float64)
    Wv64 = np.asarray(Wv, dtype=np.float64)
    Wo64 = np.asarray(Wo, dtype=np.float64)
    bv64 = np.asarray(bv, dtype=np.float64)
    bo64 = np.asarray(bo, dtype=np.float64)

    qkT = ((qe @ Wk64) / np.sqrt(float(D))).T.astype(np.float32).copy()
    m2t = (Wo64 @ Wv64).T.astype(np.float32).copy()
    g0 = (float(K) * (Wo64 @ bv64 + bo64)).astype(np.float32)
    g0c = np.ascontiguousarray(g0.reshape(2, P).T)
    ident = np.eye(P, dtype=np.float32)

    return [
        {
            "x": np.ascontiguousarray(x[b].reshape(C, HW)),
            "qkT": qkT,
            "m2t": m2t,
            "g0": g0c,
            "ident": ident,
        }
        for b in range(B)
    ]


def kernel(x, query_embed, Wk, bk, Wv, bv, Wo, bo, _trace=False, **kw):
    in_maps = make_in_maps(x, query_embed, Wk, bk, Wv, bv, Wo, bo)
    nc = _get_nc()
    res = run_bass_kernel_spmd(nc, in_maps, core_ids=list(range(B)), trace=_trace, **kw)
    out = np.stack(
        [res.results[b]["out"].reshape(C, 128, 128) for b in range(B)]
    ).astype(np.float32)
    if _trace:
        kernel.last_results = res
    return out
